# revision 12
# baseline (speedup 1.0000x reference)
"""Trainium2 Bass kernel for nn_CNF1D: 1-D continuous normalizing flow.

Reference computation (per sample b, D=1, H=256, RK4 with 4 steps over [0,1]):
    f(t,z):  h1 = tanh(z*W1[0] + t*W1[1] + b1); h2 = tanh(h1@W2 + b2);
             f = h2@W3 + b3
    JVP:     s1 = 1-h1^2;  g2 = (1-h2^2) * ((s1*W1[0])@W2);  df = g2@W3
    (z, div) integrated with RK4; outputs (z_final, div_integral).

Strategy: pure data parallelism over 8 cores (4096 samples each), 8 chunks
of 512 samples per core. Hidden-major layout ([hidden, batch]); the hidden
dim lives on SBUF partitions so biases/scales are per-partition scalars and
no transposes are needed anywhere.

Per-core state is kept in per-chunk staging tiles T [64, 512] (fp32r):
    row 0: z     rows 1-4: k1z..k4z    row 5: ones
    row 32: div  rows 33-36: kd1..kd4
The RK4 stage update z_s = z + c*dt*k_{s} is folded into the input-layer
matmul as extra contraction rows (K=6, per-eval host-built weights, with
b3 folded into the ones-row).  The RK4 combine is a K=6/K=5 matmul with
weights [1, dt/6, dt/3, dt/3, dt/6(, dt*b3)].  Stage outputs f/df are
produced by M=1 matmuls into PSUM partitions 0/32 (tile_position col
tiling), evacuated [64,512] by DVE (quadrant rule), and routed to the
right T rows by an SBUF->SBUF DMA gather (only DMA may remap partitions).

dtypes: state rows + input/combine matmuls in float32r (11 mantissa bits,
full PE speed); activations + layer-2/output matmuls in bf16 (fp32 PSUM
accumulation); tanh on ScalarE in fp32 from PSUM.
"""

import sys

for _p in ("/opt/trn_rl_repo",):
    if _p not in sys.path:
        sys.path.insert(0, _p)

import numpy as np
import ml_dtypes

import concourse.mybir as mybir
from concourse import bacc, tile
from concourse.bass_utils import run_bass_kernel_spmd

F32 = mybir.dt.float32
F32R = mybir.dt.float32r
BF16 = mybir.dt.bfloat16
ALU = mybir.AluOpType
TANH = mybir.ActivationFunctionType.Tanh

N_CORES = 8
B_TOT = 32768
B = B_TOT // N_CORES        # 4096 per core
H = 256                     # hidden
CH = 512                    # chunk (matmul N / psum bank)
NCH = B // CH               # 8 chunks per core
N_STEPS = 4
DT = 1.0 / N_STEPS
N_EVALS = 4 * N_STEPS       # 16
STAGE_OFF = [0.0, DT / 2, DT / 2, DT]
STAGE_C = [0.0, DT / 2, DT / 2, DT]


def _f32r(x):
    """Round to fp32r (11 explicit mantissa bits, RNE) to match what the
    hardware consumes; keeps host preprocessing consistent with PE."""
    b = np.ascontiguousarray(np.asarray(x, np.float32)).view(np.uint32)
    r = (b + np.uint32(0x7FF) + ((b >> np.uint32(12)) & np.uint32(1))) & np.uint32(
        0xFFFFF000
    )
    return r.view(np.float32).copy()


def _build_nc():
    nc = bacc.Bacc("TRN2", target_bir_lowering=False, debug=False,
                   num_devices=N_CORES)

    t0u = nc.dram_tensor("t0u", (NCH, 11, CH), F32R, kind="ExternalInput")
    lin = nc.dram_tensor("lin", (6, N_EVALS * H), F32R, kind="ExternalInput")
    combzd = nc.dram_tensor("combzd", (11, 2), F32R, kind="ExternalInput")
    w2 = nc.dram_tensor("w2", (128, 512), BF16, kind="ExternalInput")
    w2gn = nc.dram_tensor("w2gn", (128, 512), BF16, kind="ExternalInput")
    w3 = nc.dram_tensor("w3", (128, 2), BF16, kind="ExternalInput")
    c2 = nc.dram_tensor("c2", (128, 2), F32, kind="ExternalInput")
    b2 = nc.dram_tensor("b2", (128, 2), F32, kind="ExternalInput")

    zf = nc.dram_tensor("zf", (NCH, CH), F32R, kind="ExternalOutput")
    dv = nc.dram_tensor("dv", (NCH, CH), F32R, kind="ExternalOutput")

    with tile.TileContext(nc) as tc:
        with (
            tc.tile_pool(name="const", bufs=1) as cpool,
            tc.tile_pool(name="state", bufs=1) as spool,
            tc.tile_pool(name="work", bufs=12) as wpool,
            tc.tile_pool(name="psum", bufs=2, space="PSUM") as ppool,
            tc.tile_pool(name="psum4", bufs=4, space="PSUM") as ppool4,
        ):
            lint = cpool.tile([6, N_EVALS * H], F32R)
            combt = cpool.tile([11, 2], F32R)
            w2t = cpool.tile([128, 512], BF16)
            w2gnt = cpool.tile([128, 512], BF16)
            w3t = cpool.tile([128, 2], BF16)
            c2t = cpool.tile([128, 2], F32)
            b2t = cpool.tile([128, 2], F32)
            nc.sync.dma_start(lint[:], lin[:])
            nc.sync.dma_start(combt[:], combzd[:])
            nc.sync.dma_start(w2t[:], w2[:])
            nc.sync.dma_start(w2gnt[:], w2gn[:])
            nc.sync.dma_start(w3t[:], w3[:])
            nc.sync.dma_start(c2t[:], c2[:])
            nc.sync.dma_start(b2t[:], b2[:])

            U = []
            for c in range(NCH):
                u = spool.tile([11, CH], F32R, tag=f"U{c}")
                nc.sync.dma_start(u[:], t0u[c, :, :])
                U.append(u)

            for e in range(N_EVALS):
                s = e % 4
                for c in range(NCH):
                    Uc = U[c]
                    # input layer: K=6 matmul over [z, k1..k4, ones]; both
                    # Mtiles land in one [128,1024] psum tile -> one wide tanh
                    h1 = wpool.tile([128, 2 * CH], BF16, tag="h1")
                    for m in range(2):
                        pre1 = ppool.tile([128, CH], F32, tag="pre1")
                        nc.tensor.matmul(
                            pre1[:],
                            lint[:, e * H + m * 128 : e * H + (m + 1) * 128],
                            Uc[0:6, :],
                        )
                        nc.scalar.activation(
                            h1[:, m * CH : (m + 1) * CH], pre1[:], TANH
                        )
                    sq1 = wpool.tile([128, 2 * CH], BF16, tag="sq1")
                    nc.vector.tensor_tensor(sq1[:], h1[:], h1[:], ALU.mult)
                    # layer 2: h-stream (W2) and g-stream (-W2g, rhs=h1^2)
                    h2 = wpool.tile([128, 2 * CH], BF16, tag="h2")
                    g2ps = []
                    for mo in range(2):
                        a2 = ppool4.tile([128, CH], F32, tag="mm2")
                        for k in range(2):
                            nc.tensor.matmul(
                                a2[:],
                                w2t[:, k * 256 + mo * 128 : k * 256 + (mo + 1) * 128],
                                h1[:, k * CH : (k + 1) * CH],
                                start=(k == 0),
                                stop=(k == 1),
                            )
                        nc.scalar.activation(
                            h2[:, mo * CH : (mo + 1) * CH], a2[:], TANH,
                            bias=b2t[:, mo : mo + 1],
                        )
                        g2p = ppool4.tile([128, CH], F32, tag="mm2")
                        for k in range(2):
                            nc.tensor.matmul(
                                g2p[:],
                                w2gnt[:, k * 256 + mo * 128 : k * 256 + (mo + 1) * 128],
                                sq1[:, k * CH : (k + 1) * CH],
                                start=(k == 0),
                                stop=(k == 1),
                            )
                        g2ps.append(g2p)
                    sq2 = wpool.tile([128, 2 * CH], BF16, tag="sq2")
                    nc.vector.tensor_tensor(sq2[:], h2[:], h2[:], ALU.mult)
                    s2 = wpool.tile([128, 2 * CH], BF16, tag="s2")
                    nc.vector.tensor_scalar(s2[:], sq2[:], -1.0, 1.0, ALU.mult, ALU.add)
                    g2 = wpool.tile([128, 2 * CH], BF16, tag="g2")
                    for mo in range(2):
                        # g2 = (g2p + C2) * (1 - h2^2)
                        nc.vector.scalar_tensor_tensor(
                            g2[:, mo * CH : (mo + 1) * CH], g2ps[mo][:],
                            c2t[:, mo : mo + 1], s2[:, mo * CH : (mo + 1) * CH],
                            ALU.add, ALU.mult,
                        )
                    # output layer: f -> psum row 0, df -> psum row 32
                    coll = ppool.tile([64, CH], F32, tag="coll")
                    for k in range(2):
                        nc.tensor.matmul(
                            coll[0:1, :], w3t[:, k : k + 1],
                            h2[:, k * CH : (k + 1) * CH],
                            start=(k == 0), stop=(k == 1),
                        )
                        nc.tensor.matmul(
                            coll[32:33, :], w3t[:, k : k + 1],
                            g2[:, k * CH : (k + 1) * CH],
                            start=(k == 0), stop=(k == 1),
                            tile_position=(0, 32),
                        )
                    scr = wpool.tile([64, CH], F32R, tag="scr")
                    if c % 2 == 0:
                        nc.vector.tensor_copy(scr[:], coll[:])
                    else:
                        nc.scalar.activation(
                            scr[:], coll[:], mybir.ActivationFunctionType.Copy
                        )
                    # route f -> U[1+s], df -> U[7+s] (one strided DMA)
                    dma_eng = nc.sync if c % 2 == 0 else nc.gpsimd
                    dma_eng.dma_start(Uc[1 + s : 8 + s : 6, :], scr[0:33:32, :])
                    if s == 3:
                        # RK4 combine: one K=11 M=2 matmul -> [z_new; div_new]
                        cc = ppool.tile([64, CH], F32, tag="coll")
                        nc.tensor.matmul(cc[0:2, :], combt[:], Uc[0:11, :])
                        scr2 = wpool.tile([64, CH], F32R, tag="scr")
                        nc.vector.tensor_copy(scr2[:], cc[:])
                        nc.sync.dma_start(Uc[0:7:6, :], scr2[0:2, :])

            for c in range(NCH):
                nc.sync.dma_start(zf[c : c + 1, :], U[c][0:1, :])
                nc.sync.dma_start(dv[c : c + 1, :], U[c][6:7, :])

    nc.compile()
    return nc


_NC_CACHE = None


def _get_nc():
    global _NC_CACHE
    if _NC_CACHE is None:
        _NC_CACHE = _build_nc()
    return _NC_CACHE


def _host_prep(z0, W1, b1, W2, b2, W3, b3):
    """Build per-core input maps (host-side folds; all tiny)."""
    z0 = np.asarray(z0, np.float32)
    W1 = np.asarray(W1, np.float32)
    b1 = np.asarray(b1, np.float32)
    W2 = np.asarray(W2, np.float32)
    b2v = np.asarray(b2, np.float32)
    W3 = np.asarray(W3, np.float32)
    b3v = float(np.asarray(b3, np.float32).reshape(()))

    w1r0, w1r1 = W1[0], W1[1]

    lin = np.zeros((6, N_EVALS * H), np.float32)
    for e in range(N_EVALS):
        i, s = divmod(e, 4)
        t_e = i * DT + STAGE_OFF[s]
        c_e = STAGE_C[s]
        blk = lin[:, e * H : (e + 1) * H]
        blk[0] = w1r0
        if s >= 1:
            blk[s] = c_e * w1r0
        blk[5] = t_e * w1r1 + b1 + c_e * b3v * w1r0
    combzd = np.zeros((11, 2), np.float32)
    combzd[:, 0] = [1.0, DT / 6, DT / 3, DT / 3, DT / 6, DT * b3v, 0, 0, 0, 0, 0]
    combzd[:, 1] = [0, 0, 0, 0, 0, 0, 1.0, DT / 6, DT / 3, DT / 3, DT / 6]

    w2p = np.concatenate([W2[0:128, :], W2[128:256, :]], axis=1)  # [128,512]
    w2g = W2 * w1r0[:, None]
    w2gnp = np.concatenate([-w2g[0:128, :], -w2g[128:256, :]], axis=1)
    c2 = w2g.sum(axis=0)  # [256]
    c2p = np.stack([c2[0:128], c2[128:256]], axis=1)  # [128,2]
    b2p = np.stack([b2v[0:128], b2v[128:256]], axis=1)
    w3p = np.stack([W3[0:128, 0], W3[128:256, 0]], axis=1)  # [128,2]

    shared = {
        "lin": _f32r(lin),
        "combzd": _f32r(combzd),
        "w2": w2p.astype(ml_dtypes.bfloat16),
        "w2gn": w2gnp.astype(ml_dtypes.bfloat16),
        "w3": w3p.astype(ml_dtypes.bfloat16),
        "c2": c2p,
        "b2": b2p,
    }
    in_maps = []
    for core in range(N_CORES):
        zc = z0[core * B : (core + 1) * B, 0].reshape(NCH, CH)
        t0uv = np.zeros((NCH, 11, CH), np.float32)
        t0uv[:, 0, :] = _f32r(zc)
        t0uv[:, 5, :] = 1.0
        in_maps.append({"t0u": t0uv, **shared})
    return in_maps


def _run(in_maps, **kw):
    nc = _get_nc()
    return run_bass_kernel_spmd(nc, in_maps, core_ids=list(range(N_CORES)), **kw)


def kernel(z0, W1, b1, W2, b2, W3, b3):
    in_maps = _host_prep(z0, W1, b1, W2, b2, W3, b3)
    res = _run(in_maps)
    zf = np.concatenate(
        [np.asarray(r["zf"], np.float32).reshape(B, 1) for r in res.results]
    )
    dv = np.concatenate(
        [np.asarray(r["dv"], np.float32).reshape(B, 1) for r in res.results]
    )
    return zf, dv


# revision 13
# speedup vs baseline: 1.0354x; 1.0354x over previous
"""Trainium2 Bass kernel for nn_CNF1D: 1-D continuous normalizing flow.

Reference computation (per sample b, D=1, H=256, RK4 with 4 steps over [0,1]):
    f(t,z):  h1 = tanh(z*W1[0] + t*W1[1] + b1); h2 = tanh(h1@W2 + b2);
             f = h2@W3 + b3
    JVP:     s1 = 1-h1^2;  g2 = (1-h2^2) * ((s1*W1[0])@W2);  df = g2@W3
    (z, div) integrated with RK4; outputs (z_final, div_integral).

Strategy: pure data parallelism over 8 cores (4096 samples each), 8 chunks
of 512 samples per core. Hidden-major layout ([hidden, batch]); the hidden
dim lives on SBUF partitions so biases/scales are per-partition scalars and
no transposes are needed anywhere.

Per-core state is kept in per-chunk staging tiles T [64, 512] (fp32r):
    row 0: z     rows 1-4: k1z..k4z    row 5: ones
    row 32: div  rows 33-36: kd1..kd4
The RK4 stage update z_s = z + c*dt*k_{s} is folded into the input-layer
matmul as extra contraction rows (K=6, per-eval host-built weights, with
b3 folded into the ones-row).  The RK4 combine is a K=6/K=5 matmul with
weights [1, dt/6, dt/3, dt/3, dt/6(, dt*b3)].  Stage outputs f/df are
produced by M=1 matmuls into PSUM partitions 0/32 (tile_position col
tiling), evacuated [64,512] by DVE (quadrant rule), and routed to the
right T rows by an SBUF->SBUF DMA gather (only DMA may remap partitions).

dtypes: state rows + input/combine matmuls in float32r (11 mantissa bits,
full PE speed); activations + layer-2/output matmuls in bf16 (fp32 PSUM
accumulation); tanh on ScalarE in fp32 from PSUM.
"""

import sys

for _p in ("/opt/trn_rl_repo",):
    if _p not in sys.path:
        sys.path.insert(0, _p)

import numpy as np
import ml_dtypes

import concourse.mybir as mybir
from concourse import bacc, tile
from concourse.bass_utils import run_bass_kernel_spmd

F32 = mybir.dt.float32
F32R = mybir.dt.float32r
BF16 = mybir.dt.bfloat16
ALU = mybir.AluOpType
TANH = mybir.ActivationFunctionType.Tanh

N_CORES = 8
B_TOT = 32768
B = B_TOT // N_CORES        # 4096 per core
H = 256                     # hidden
CH = 512                    # chunk (matmul N / psum bank)
NCH = B // CH               # 8 chunks per core
N_STEPS = 4
DT = 1.0 / N_STEPS
N_EVALS = 4 * N_STEPS       # 16
STAGE_OFF = [0.0, DT / 2, DT / 2, DT]
STAGE_C = [0.0, DT / 2, DT / 2, DT]


def _f32r(x):
    """Round to fp32r (11 explicit mantissa bits, RNE) to match what the
    hardware consumes; keeps host preprocessing consistent with PE."""
    b = np.ascontiguousarray(np.asarray(x, np.float32)).view(np.uint32)
    r = (b + np.uint32(0x7FF) + ((b >> np.uint32(12)) & np.uint32(1))) & np.uint32(
        0xFFFFF000
    )
    return r.view(np.float32).copy()


def _build_nc():
    nc = bacc.Bacc("TRN2", target_bir_lowering=False, debug=False,
                   num_devices=N_CORES)

    t0u = nc.dram_tensor("t0u", (NCH, 11, CH), F32R, kind="ExternalInput")
    lin = nc.dram_tensor("lin", (6, N_EVALS * H), F32R, kind="ExternalInput")
    combzd = nc.dram_tensor("combzd", (11, 2), F32R, kind="ExternalInput")
    w2 = nc.dram_tensor("w2", (128, 512), BF16, kind="ExternalInput")
    w2gn = nc.dram_tensor("w2gn", (128, 512), BF16, kind="ExternalInput")
    w3 = nc.dram_tensor("w3", (128, 2), BF16, kind="ExternalInput")
    c2 = nc.dram_tensor("c2", (128, 2), F32, kind="ExternalInput")
    b2 = nc.dram_tensor("b2", (128, 2), F32, kind="ExternalInput")

    zf = nc.dram_tensor("zf", (NCH, CH), F32R, kind="ExternalOutput")
    dv = nc.dram_tensor("dv", (NCH, CH), F32R, kind="ExternalOutput")

    with tile.TileContext(nc) as tc:
        with (
            tc.tile_pool(name="const", bufs=1) as cpool,
            tc.tile_pool(name="state", bufs=1) as spool,
            tc.tile_pool(name="work", bufs=12) as wpool,
            tc.tile_pool(name="psum", bufs=2, space="PSUM") as ppool,
        ):
            lint = cpool.tile([6, N_EVALS * H], F32R)
            combt = cpool.tile([11, 2], F32R)
            w2t = cpool.tile([128, 512], BF16)
            w2gnt = cpool.tile([128, 512], BF16)
            w3t = cpool.tile([128, 2], BF16)
            c2t = cpool.tile([128, 2], F32)
            b2t = cpool.tile([128, 2], F32)
            nc.sync.dma_start(lint[:], lin[:])
            nc.sync.dma_start(combt[:], combzd[:])
            nc.sync.dma_start(w2t[:], w2[:])
            nc.sync.dma_start(w2gnt[:], w2gn[:])
            nc.sync.dma_start(w3t[:], w3[:])
            nc.sync.dma_start(c2t[:], c2[:])
            nc.sync.dma_start(b2t[:], b2[:])

            U = []
            for c in range(NCH):
                u = spool.tile([11, CH], F32R, tag=f"U{c}")
                nc.sync.dma_start(u[:], t0u[c, :, :])
                U.append(u)

            for e in range(N_EVALS):
                s = e % 4
                for c in range(NCH):
                    Uc = U[c]
                    # input layer: K=6 matmul over [z, k1..k4, ones]; both
                    # Mtiles land in one [128,1024] psum tile -> one wide tanh
                    h1 = wpool.tile([128, 2 * CH], BF16, tag="h1")
                    for m in range(2):
                        pre1 = ppool.tile([128, CH], F32, tag="pre1")
                        nc.tensor.matmul(
                            pre1[:],
                            lint[:, e * H + m * 128 : e * H + (m + 1) * 128],
                            Uc[0:6, :],
                        )
                        nc.scalar.activation(
                            h1[:, m * CH : (m + 1) * CH], pre1[:], TANH
                        )
                    sq1 = wpool.tile([128, 2 * CH], BF16, tag="sq1")
                    nc.vector.tensor_tensor(sq1[:], h1[:], h1[:], ALU.mult)
                    # layer 2: h-stream (W2) and g-stream (-W2g, rhs=h1^2)
                    h2 = wpool.tile([128, 2 * CH], BF16, tag="h2")
                    g2ps = []
                    for mo in range(2):
                        a2 = ppool.tile([128, CH], F32, tag="a2")
                        for k in range(2):
                            nc.tensor.matmul(
                                a2[:],
                                w2t[:, k * 256 + mo * 128 : k * 256 + (mo + 1) * 128],
                                h1[:, k * CH : (k + 1) * CH],
                                start=(k == 0),
                                stop=(k == 1),
                            )
                        nc.scalar.activation(
                            h2[:, mo * CH : (mo + 1) * CH], a2[:], TANH,
                            bias=b2t[:, mo : mo + 1],
                        )
                        g2p = ppool.tile([128, CH], F32, tag="g2p")
                        for k in range(2):
                            nc.tensor.matmul(
                                g2p[:],
                                w2gnt[:, k * 256 + mo * 128 : k * 256 + (mo + 1) * 128],
                                sq1[:, k * CH : (k + 1) * CH],
                                start=(k == 0),
                                stop=(k == 1),
                            )
                        g2ps.append(g2p)
                    sq2 = wpool.tile([128, 2 * CH], BF16, tag="sq2")
                    nc.vector.tensor_tensor(sq2[:], h2[:], h2[:], ALU.mult)
                    s2 = wpool.tile([128, 2 * CH], BF16, tag="s2")
                    nc.vector.tensor_scalar(s2[:], sq2[:], -1.0, 1.0, ALU.mult, ALU.add)
                    g2 = wpool.tile([128, 2 * CH], BF16, tag="g2")
                    for mo in range(2):
                        # g2 = (g2p + C2) * (1 - h2^2)
                        nc.vector.scalar_tensor_tensor(
                            g2[:, mo * CH : (mo + 1) * CH], g2ps[mo][:],
                            c2t[:, mo : mo + 1], s2[:, mo * CH : (mo + 1) * CH],
                            ALU.add, ALU.mult,
                        )
                    # output layer: f -> psum row 0, df -> psum row 32
                    coll = ppool.tile([64, CH], F32, tag="coll")
                    for k in range(2):
                        nc.tensor.matmul(
                            coll[0:1, :], w3t[:, k : k + 1],
                            h2[:, k * CH : (k + 1) * CH],
                            start=(k == 0), stop=(k == 1),
                        )
                        nc.tensor.matmul(
                            coll[32:33, :], w3t[:, k : k + 1],
                            g2[:, k * CH : (k + 1) * CH],
                            start=(k == 0), stop=(k == 1),
                            tile_position=(0, 32),
                        )
                    scr = wpool.tile([64, CH], F32R, tag="scr")
                    if c % 2 == 0:
                        nc.vector.tensor_copy(scr[:], coll[:])
                    else:
                        nc.scalar.activation(
                            scr[:], coll[:], mybir.ActivationFunctionType.Copy
                        )
                    # route f -> U[1+s], df -> U[7+s] (one strided DMA)
                    dma_eng = nc.sync if c % 2 == 0 else nc.gpsimd
                    dma_eng.dma_start(Uc[1 + s : 8 + s : 6, :], scr[0:33:32, :])
                    if s == 3:
                        # RK4 combine: one K=11 M=2 matmul -> [z_new; div_new]
                        cc = ppool.tile([64, CH], F32, tag="coll")
                        nc.tensor.matmul(cc[0:2, :], combt[:], Uc[0:11, :])
                        scr2 = wpool.tile([64, CH], F32R, tag="scr")
                        nc.vector.tensor_copy(scr2[:], cc[:])
                        nc.sync.dma_start(Uc[0:7:6, :], scr2[0:2, :])

            for c in range(NCH):
                nc.sync.dma_start(zf[c : c + 1, :], U[c][0:1, :])
                nc.sync.dma_start(dv[c : c + 1, :], U[c][6:7, :])

    nc.compile()
    return nc


_NC_CACHE = None


def _get_nc():
    global _NC_CACHE
    if _NC_CACHE is None:
        _NC_CACHE = _build_nc()
    return _NC_CACHE


def _host_prep(z0, W1, b1, W2, b2, W3, b3):
    """Build per-core input maps (host-side folds; all tiny)."""
    z0 = np.asarray(z0, np.float32)
    W1 = np.asarray(W1, np.float32)
    b1 = np.asarray(b1, np.float32)
    W2 = np.asarray(W2, np.float32)
    b2v = np.asarray(b2, np.float32)
    W3 = np.asarray(W3, np.float32)
    b3v = float(np.asarray(b3, np.float32).reshape(()))

    w1r0, w1r1 = W1[0], W1[1]

    lin = np.zeros((6, N_EVALS * H), np.float32)
    for e in range(N_EVALS):
        i, s = divmod(e, 4)
        t_e = i * DT + STAGE_OFF[s]
        c_e = STAGE_C[s]
        blk = lin[:, e * H : (e + 1) * H]
        blk[0] = w1r0
        if s >= 1:
            blk[s] = c_e * w1r0
        blk[5] = t_e * w1r1 + b1 + c_e * b3v * w1r0
    combzd = np.zeros((11, 2), np.float32)
    combzd[:, 0] = [1.0, DT / 6, DT / 3, DT / 3, DT / 6, DT * b3v, 0, 0, 0, 0, 0]
    combzd[:, 1] = [0, 0, 0, 0, 0, 0, 1.0, DT / 6, DT / 3, DT / 3, DT / 6]

    w2p = np.concatenate([W2[0:128, :], W2[128:256, :]], axis=1)  # [128,512]
    w2g = W2 * w1r0[:, None]
    w2gnp = np.concatenate([-w2g[0:128, :], -w2g[128:256, :]], axis=1)
    c2 = w2g.sum(axis=0)  # [256]
    c2p = np.stack([c2[0:128], c2[128:256]], axis=1)  # [128,2]
    b2p = np.stack([b2v[0:128], b2v[128:256]], axis=1)
    w3p = np.stack([W3[0:128, 0], W3[128:256, 0]], axis=1)  # [128,2]

    shared = {
        "lin": _f32r(lin),
        "combzd": _f32r(combzd),
        "w2": w2p.astype(ml_dtypes.bfloat16),
        "w2gn": w2gnp.astype(ml_dtypes.bfloat16),
        "w3": w3p.astype(ml_dtypes.bfloat16),
        "c2": c2p,
        "b2": b2p,
    }
    in_maps = []
    for core in range(N_CORES):
        zc = z0[core * B : (core + 1) * B, 0].reshape(NCH, CH)
        t0uv = np.zeros((NCH, 11, CH), np.float32)
        t0uv[:, 0, :] = _f32r(zc)
        t0uv[:, 5, :] = 1.0
        in_maps.append({"t0u": t0uv, **shared})
    return in_maps


def _run(in_maps, **kw):
    nc = _get_nc()
    return run_bass_kernel_spmd(nc, in_maps, core_ids=list(range(N_CORES)), **kw)


def kernel(z0, W1, b1, W2, b2, W3, b3):
    in_maps = _host_prep(z0, W1, b1, W2, b2, W3, b3)
    res = _run(in_maps)
    zf = np.concatenate(
        [np.asarray(r["zf"], np.float32).reshape(B, 1) for r in res.results]
    )
    dv = np.concatenate(
        [np.asarray(r["dv"], np.float32).reshape(B, 1) for r in res.results]
    )
    return zf, dv


# revision 14
# speedup vs baseline: 1.1125x; 1.0745x over previous
"""Trainium2 Bass kernel for nn_CNF1D: 1-D continuous normalizing flow.

Reference computation (per sample b, D=1, H=256, RK4 with 4 steps over [0,1]):
    f(t,z):  h1 = tanh(z*W1[0] + t*W1[1] + b1); h2 = tanh(h1@W2 + b2);
             f = h2@W3 + b3
    JVP:     s1 = 1-h1^2;  g2 = (1-h2^2) * ((s1*W1[0])@W2);  df = g2@W3
    (z, div) integrated with RK4; outputs (z_final, div_integral).

Strategy: pure data parallelism over 8 cores (4096 samples each), 8 chunks
of 512 samples per core. Hidden-major layout ([hidden, batch]); the hidden
dim lives on SBUF partitions so biases/scales are per-partition scalars and
no transposes are needed anywhere.

Per-core state is kept in per-chunk staging tiles T [64, 512] (fp32r):
    row 0: z     rows 1-4: k1z..k4z    row 5: ones
    row 32: div  rows 33-36: kd1..kd4
The RK4 stage update z_s = z + c*dt*k_{s} is folded into the input-layer
matmul as extra contraction rows (K=6, per-eval host-built weights, with
b3 folded into the ones-row).  The RK4 combine is a K=6/K=5 matmul with
weights [1, dt/6, dt/3, dt/3, dt/6(, dt*b3)].  Stage outputs f/df are
produced by M=1 matmuls into PSUM partitions 0/32 (tile_position col
tiling), evacuated [64,512] by DVE (quadrant rule), and routed to the
right T rows by an SBUF->SBUF DMA gather (only DMA may remap partitions).

dtypes: state rows + input/combine matmuls in float32r (11 mantissa bits,
full PE speed); activations + layer-2/output matmuls in bf16 (fp32 PSUM
accumulation); tanh on ScalarE in fp32 from PSUM.
"""

import sys

for _p in ("/opt/trn_rl_repo",):
    if _p not in sys.path:
        sys.path.insert(0, _p)

import numpy as np
import ml_dtypes

import concourse.mybir as mybir
from concourse import bacc, tile
from concourse.bass_utils import run_bass_kernel_spmd

F32 = mybir.dt.float32
F32R = mybir.dt.float32r
BF16 = mybir.dt.bfloat16
ALU = mybir.AluOpType
TANH = mybir.ActivationFunctionType.Tanh

N_CORES = 8
B_TOT = 32768
B = B_TOT // N_CORES        # 4096 per core
H = 256                     # hidden
CH = 512                    # chunk (matmul N / psum bank)
NCH = B // CH               # 8 chunks per core
N_STEPS = 4
DT = 1.0 / N_STEPS
N_EVALS = 4 * N_STEPS       # 16
STAGE_OFF = [0.0, DT / 2, DT / 2, DT]
STAGE_C = [0.0, DT / 2, DT / 2, DT]


def _f32r(x):
    """Round to fp32r (11 explicit mantissa bits, RNE) to match what the
    hardware consumes; keeps host preprocessing consistent with PE."""
    b = np.ascontiguousarray(np.asarray(x, np.float32)).view(np.uint32)
    r = (b + np.uint32(0x7FF) + ((b >> np.uint32(12)) & np.uint32(1))) & np.uint32(
        0xFFFFF000
    )
    return r.view(np.float32).copy()


def _build_nc():
    nc = bacc.Bacc("TRN2", target_bir_lowering=False, debug=False,
                   num_devices=N_CORES)

    t0u = nc.dram_tensor("t0u", (NCH, 11, CH), F32R, kind="ExternalInput")
    lin = nc.dram_tensor("lin", (6, N_EVALS * H), F32R, kind="ExternalInput")
    combzd = nc.dram_tensor("combzd", (11, 2), F32R, kind="ExternalInput")
    w2 = nc.dram_tensor("w2", (128, 512), BF16, kind="ExternalInput")
    w2gn = nc.dram_tensor("w2gn", (128, 512), BF16, kind="ExternalInput")
    w3 = nc.dram_tensor("w3", (128, 2), BF16, kind="ExternalInput")
    c2 = nc.dram_tensor("c2", (128, 2), F32, kind="ExternalInput")
    b2 = nc.dram_tensor("b2", (128, 2), F32, kind="ExternalInput")

    zf = nc.dram_tensor("zf", (NCH, CH), F32R, kind="ExternalOutput")
    dv = nc.dram_tensor("dv", (NCH, CH), F32R, kind="ExternalOutput")

    with tile.TileContext(nc) as tc:
        with (
            tc.tile_pool(name="const", bufs=1) as cpool,
            tc.tile_pool(name="state", bufs=1) as spool,
            tc.tile_pool(name="work", bufs=12) as wpool,
            tc.tile_pool(name="psum", bufs=2, space="PSUM") as ppool,
        ):
            lint = cpool.tile([6, N_EVALS * H], F32R)
            combt = cpool.tile([11, 2], F32R)
            w2t = cpool.tile([128, 512], BF16)
            w2gnt = cpool.tile([128, 512], BF16)
            w3t = cpool.tile([128, 2], BF16)
            c2t = cpool.tile([128, 2], F32)
            b2t = cpool.tile([128, 2], F32)
            nc.sync.dma_start(lint[:], lin[:])
            nc.sync.dma_start(combt[:], combzd[:])
            nc.sync.dma_start(w2t[:], w2[:])
            nc.sync.dma_start(w2gnt[:], w2gn[:])
            nc.sync.dma_start(w3t[:], w3[:])
            nc.sync.dma_start(c2t[:], c2[:])
            nc.sync.dma_start(b2t[:], b2[:])

            U = []
            for c in range(NCH):
                u = spool.tile([11, CH], F32R, tag=f"U{c}")
                nc.sync.dma_start(u[:], t0u[c, :, :])
                U.append(u)

            for e in range(N_EVALS):
                s = e % 4
                for cp in range(NCH // 2):
                    pair_h2g2 = []
                    for ci in range(2):
                        c = 2 * cp + ci
                        Uc = U[c]
                        # input layer: K=6 matmul over [z, k1..k4, ones]
                        h1 = wpool.tile([128, 2 * CH], BF16, tag="h1")
                        for m in range(2):
                            pre1 = ppool.tile([128, CH], F32, tag="pre1")
                            nc.tensor.matmul(
                                pre1[:],
                                lint[:, e * H + m * 128 : e * H + (m + 1) * 128],
                                Uc[0:6, :],
                            )
                            nc.scalar.activation(
                                h1[:, m * CH : (m + 1) * CH], pre1[:], TANH
                            )
                        sq1 = wpool.tile([128, 2 * CH], BF16, tag="sq1")
                        nc.vector.tensor_tensor(sq1[:], h1[:], h1[:], ALU.mult)
                        # layer 2: h-stream (W2) and g-stream (-W2g, rhs=h1^2)
                        h2 = wpool.tile([128, 2 * CH], BF16, tag="h2")
                        g2ps = []
                        for mo in range(2):
                            a2 = ppool.tile([128, CH], F32, tag="a2")
                            for k in range(2):
                                nc.tensor.matmul(
                                    a2[:],
                                    w2t[:, k * 256 + mo * 128 : k * 256 + (mo + 1) * 128],
                                    h1[:, k * CH : (k + 1) * CH],
                                    start=(k == 0),
                                    stop=(k == 1),
                                )
                            nc.scalar.activation(
                                h2[:, mo * CH : (mo + 1) * CH], a2[:], TANH,
                                bias=b2t[:, mo : mo + 1],
                            )
                            g2p = ppool.tile([128, CH], F32, tag="g2p")
                            for k in range(2):
                                nc.tensor.matmul(
                                    g2p[:],
                                    w2gnt[:, k * 256 + mo * 128 : k * 256 + (mo + 1) * 128],
                                    sq1[:, k * CH : (k + 1) * CH],
                                    start=(k == 0),
                                    stop=(k == 1),
                                )
                            g2ps.append(g2p)
                        sq2 = wpool.tile([128, 2 * CH], BF16, tag="sq2")
                        nc.vector.tensor_tensor(sq2[:], h2[:], h2[:], ALU.mult)
                        s2 = wpool.tile([128, 2 * CH], BF16, tag="s2")
                        nc.vector.tensor_scalar(s2[:], sq2[:], -1.0, 1.0, ALU.mult, ALU.add)
                        g2 = wpool.tile([128, 2 * CH], BF16, tag="g2")
                        for mo in range(2):
                            # g2 = (g2p + C2) * (1 - h2^2)
                            nc.vector.scalar_tensor_tensor(
                                g2[:, mo * CH : (mo + 1) * CH], g2ps[mo][:],
                                c2t[:, mo : mo + 1], s2[:, mo * CH : (mo + 1) * CH],
                                ALU.add, ALU.mult,
                            )
                        pair_h2g2.append((h2, g2))
                    # output layer for BOTH chunks into one collector:
                    # chunk ci: f -> partition 64*ci, df -> partition 64*ci+32
                    coll = ppool.tile([128, CH], F32, tag="coll")
                    for k in range(2):
                        for ci in range(2):
                            h2, g2 = pair_h2g2[ci]
                            pf = 64 * ci
                            nc.tensor.matmul(
                                coll[pf : pf + 1, :], w3t[:, k : k + 1],
                                h2[:, k * CH : (k + 1) * CH],
                                start=(k == 0), stop=(k == 1),
                                tile_position=(0, pf),
                            )
                            nc.tensor.matmul(
                                coll[pf + 32 : pf + 33, :], w3t[:, k : k + 1],
                                g2[:, k * CH : (k + 1) * CH],
                                start=(k == 0), stop=(k == 1),
                                tile_position=(0, pf + 32),
                            )
                    scr = wpool.tile([128, CH], F32R, tag="scr")
                    if cp % 2 == 0:
                        nc.vector.tensor_copy(scr[:], coll[:])
                    else:
                        nc.scalar.activation(
                            scr[:], coll[:], mybir.ActivationFunctionType.Copy
                        )
                    for ci in range(2):
                        c = 2 * cp + ci
                        dma_eng = nc.sync if ci == 0 else nc.gpsimd
                        dma_eng.dma_start(
                            U[c][1 + s : 8 + s : 6, :],
                            scr[64 * ci : 64 * ci + 33 : 32, :],
                        )
                    if s == 3:
                        for ci in range(2):
                            c = 2 * cp + ci
                            # RK4 combine: one K=11 M=2 matmul -> [z_new; div_new]
                            cc = ppool.tile([128, CH], F32, tag="coll")
                            nc.tensor.matmul(cc[0:2, :], combt[:], U[c][0:11, :])
                            scr2 = wpool.tile([128, CH], F32R, tag="scr")
                            nc.vector.tensor_copy(scr2[0:2, :], cc[0:2, :])
                            nc.sync.dma_start(U[c][0:7:6, :], scr2[0:2, :])

            for c in range(NCH):
                nc.sync.dma_start(zf[c : c + 1, :], U[c][0:1, :])
                nc.sync.dma_start(dv[c : c + 1, :], U[c][6:7, :])

    nc.compile()
    return nc


_NC_CACHE = None


def _get_nc():
    global _NC_CACHE
    if _NC_CACHE is None:
        _NC_CACHE = _build_nc()
    return _NC_CACHE


def _host_prep(z0, W1, b1, W2, b2, W3, b3):
    """Build per-core input maps (host-side folds; all tiny)."""
    z0 = np.asarray(z0, np.float32)
    W1 = np.asarray(W1, np.float32)
    b1 = np.asarray(b1, np.float32)
    W2 = np.asarray(W2, np.float32)
    b2v = np.asarray(b2, np.float32)
    W3 = np.asarray(W3, np.float32)
    b3v = float(np.asarray(b3, np.float32).reshape(()))

    w1r0, w1r1 = W1[0], W1[1]

    lin = np.zeros((6, N_EVALS * H), np.float32)
    for e in range(N_EVALS):
        i, s = divmod(e, 4)
        t_e = i * DT + STAGE_OFF[s]
        c_e = STAGE_C[s]
        blk = lin[:, e * H : (e + 1) * H]
        blk[0] = w1r0
        if s >= 1:
            blk[s] = c_e * w1r0
        blk[5] = t_e * w1r1 + b1 + c_e * b3v * w1r0
    combzd = np.zeros((11, 2), np.float32)
    combzd[:, 0] = [1.0, DT / 6, DT / 3, DT / 3, DT / 6, DT * b3v, 0, 0, 0, 0, 0]
    combzd[:, 1] = [0, 0, 0, 0, 0, 0, 1.0, DT / 6, DT / 3, DT / 3, DT / 6]

    w2p = np.concatenate([W2[0:128, :], W2[128:256, :]], axis=1)  # [128,512]
    w2g = W2 * w1r0[:, None]
    w2gnp = np.concatenate([-w2g[0:128, :], -w2g[128:256, :]], axis=1)
    c2 = w2g.sum(axis=0)  # [256]
    c2p = np.stack([c2[0:128], c2[128:256]], axis=1)  # [128,2]
    b2p = np.stack([b2v[0:128], b2v[128:256]], axis=1)
    w3p = np.stack([W3[0:128, 0], W3[128:256, 0]], axis=1)  # [128,2]

    shared = {
        "lin": _f32r(lin),
        "combzd": _f32r(combzd),
        "w2": w2p.astype(ml_dtypes.bfloat16),
        "w2gn": w2gnp.astype(ml_dtypes.bfloat16),
        "w3": w3p.astype(ml_dtypes.bfloat16),
        "c2": c2p,
        "b2": b2p,
    }
    in_maps = []
    for core in range(N_CORES):
        zc = z0[core * B : (core + 1) * B, 0].reshape(NCH, CH)
        t0uv = np.zeros((NCH, 11, CH), np.float32)
        t0uv[:, 0, :] = _f32r(zc)
        t0uv[:, 5, :] = 1.0
        in_maps.append({"t0u": t0uv, **shared})
    return in_maps


def _run(in_maps, **kw):
    nc = _get_nc()
    return run_bass_kernel_spmd(nc, in_maps, core_ids=list(range(N_CORES)), **kw)


def kernel(z0, W1, b1, W2, b2, W3, b3):
    in_maps = _host_prep(z0, W1, b1, W2, b2, W3, b3)
    res = _run(in_maps)
    zf = np.concatenate(
        [np.asarray(r["zf"], np.float32).reshape(B, 1) for r in res.results]
    )
    dv = np.concatenate(
        [np.asarray(r["dv"], np.float32).reshape(B, 1) for r in res.results]
    )
    return zf, dv
